# revision 7
# baseline (speedup 1.0000x reference)
"""Per-env MLP (EnvironVectorField) Trainium2 kernel.

Reference computation (fp32):
    x = u.reshape(B, E, D)  # B=16384, E=8 envs, D=64
    h = swish(x @ W1[e] + b1[e]); h = swish(h @ W2[e] + b2[e])
    h = swish(h @ W3[e] + b3[e]); out = h @ W4[e] + b4[e]
    return out.reshape(B*E, D)

Sharding: expert-parallel — core e computes env e entirely (u rows e::8).

Per-core design: activations are feature-major (features on SBUF
partitions, batch on the free axis) so weights are the stationary matmul
operand. The input/output transposes are done on the HOST (numpy), so the
device only runs matmuls + activations.

Dtypes: everything bf16 (weights host-converted, activations written
bf16 by the PSUM-evacuating Silu). Accumulation is fp32 in PSUM.
bf16 streams at 1 cycle/row on the PE like f32r, but halves DMA bytes
and SBUF footprint. (fp8+DoubleRow was evaluated: 2x streaming rate but
4.2e-2 rel err -- over the 2e-2 gate. bf16 measures 3.6e-3.)
"""

import sys

sys.path.insert(0, '/opt/trn_rl_repo')

from contextlib import ExitStack

import ml_dtypes
import numpy as np

import concourse.bacc as bacc
import concourse.bass as bass
import concourse.mybir as mybir
import concourse.tile as tile
from concourse.bass_utils import run_bass_kernel_spmd

F32 = mybir.dt.float32
BF16 = mybir.dt.bfloat16
SILU = mybir.ActivationFunctionType.Silu

NP_BF16 = ml_dtypes.bfloat16

N_ENV = 8
D = 64          # state dim
H = 1024        # hidden dim
B = 16384       # rows per env
NB = 1024       # batch-chunk columns per chunk
NCH = B // NB   # 16 chunks
NT = NB // 512  # 512-wide matmul n-tiles per chunk
KT = H // 128   # 8 k/m tiles of 128 over the hidden dim


def build_module(iters: int = 1, nch: int = NCH):
    nc = bacc.Bacc("TRN2", target_bir_lowering=False, num_devices=N_ENV)

    xin = nc.dram_tensor("xt", (D, nch * NB), BF16, kind="ExternalInput")
    w1d = nc.dram_tensor("w1", (D, H), BF16, kind="ExternalInput")
    w2d = nc.dram_tensor("w2", (128, KT, H), BF16, kind="ExternalInput")
    w3d = nc.dram_tensor("w3", (128, KT, H), BF16, kind="ExternalInput")
    w4d = nc.dram_tensor("w4", (128, KT, D), BF16, kind="ExternalInput")
    b1d = nc.dram_tensor("b1", (128, KT), F32, kind="ExternalInput")
    b2d = nc.dram_tensor("b2", (128, KT), F32, kind="ExternalInput")
    b3d = nc.dram_tensor("b3", (128, KT), F32, kind="ExternalInput")
    b4d = nc.dram_tensor("b4", (D, 1), F32, kind="ExternalInput")
    yout = nc.dram_tensor("y", (D, nch * NB), F32, kind="ExternalOutput")

    with tile.TileContext(nc) as tc, ExitStack() as ctx:
        wpool = ctx.enter_context(tc.tile_pool(name="wpool", bufs=1))
        mps = ctx.enter_context(tc.tile_pool(name="mps", bufs=3, space="PSUM"))
        mp4 = ctx.enter_context(tc.tile_pool(name="mp4", bufs=1, space="PSUM"))

        # persistent tiles
        xts = [wpool.tile([D, NB], BF16, name=f"xt{i}") for i in range(2)]
        w1s = wpool.tile([D, H], BF16)
        w2s = wpool.tile([128, KT, H], BF16)
        w3s = wpool.tile([128, KT, H], BF16)
        w4s = wpool.tile([128, KT, D], BF16)
        ball = wpool.tile([128, 3 * KT + 1], F32)
        hAs = [wpool.tile([128, KT, NB], BF16, name=f"hA{i}") for i in range(2)]
        hBs = [wpool.tile([128, KT, NB], BF16, name=f"hB{i}") for i in range(2)]
        hCs = [wpool.tile([128, KT, NB], BF16, name=f"hC{i}") for i in range(2)]
        oTs = [wpool.tile([D, NB], F32, name=f"oT{i}") for i in range(2)]

        # chunk-0 input first (FIFO DMA queues), then weights/biases
        nc.sync.dma_start(xts[0][:], xin[:, 0:NB])
        nc.sync.dma_start(w1s[:], w1d[:])
        nc.sync.dma_start(ball[:, 0:KT], b1d[:])
        nc.sync.dma_start(ball[:, KT:2 * KT], b2d[:])
        nc.sync.dma_start(ball[:, 2 * KT:3 * KT], b3d[:])
        nc.sync.dma_start(ball[:D, 3 * KT:3 * KT + 1], b4d[:])
        nc.sync.dma_start(w2s[:], w2d[:])
        nc.sync.dma_start(w3s[:], w3d[:])
        nc.sync.dma_start(w4s[:], w4d[:])
        b4s = ball[:D, 3 * KT:3 * KT + 1]

        def dma_in(c, it=0):
            nc.sync.dma_start(xts[c % 2][:], xin[:, c * NB:(c + 1) * NB])

        def l1_group(c, m, it=0):
            xt = xts[c % 2]
            hA = hAs[c % 2]
            pm = mps.tile([128, NB], F32, tag="mmw", name=f"p1_{it}_{c}_{m}")
            for n in range(NT):
                ns_ = slice(n * 512, (n + 1) * 512)
                nc.tensor.matmul(pm[:, ns_], w1s[:, m * 128:(m + 1) * 128],
                                 xt[:, ns_], start=True, stop=True)
            nc.scalar.activation(hA[:, m, :], pm[:], SILU,
                                 bias=ball[:, m:m + 1])

        def mid_group(li, ws, hs, hd, c, m, it=0):
            pm = mps.tile([128, NB], F32, tag="mmw", name=f"p{li}_{it}_{c}_{m}")
            for k in range(KT):
                for n in range(NT):
                    ns_ = slice(n * 512, (n + 1) * 512)
                    nc.tensor.matmul(pm[:, ns_],
                                     ws[:, k, m * 128:(m + 1) * 128],
                                     hs[:, k, ns_],
                                     start=(k == 0), stop=(k == KT - 1))
            bcol = (li - 1) * KT + m
            nc.scalar.activation(hd[:, m, :], pm[:], SILU,
                                 bias=ball[:, bcol:bcol + 1])

        def l4_chunk(c, it=0):
            hC = hCs[c % 2]
            oT = oTs[c % 2]
            p4 = mp4.tile([D, NB], F32, tag="p4", name=f"p4_{it}_{c}")
            for n in range(NT):
                ns_ = slice(n * 512, (n + 1) * 512)
                for k in range(KT):
                    nc.tensor.matmul(p4[:, ns_], w4s[:, k, :], hC[:, k, ns_],
                                     start=(k == 0), stop=(k == KT - 1))
            nc.vector.tensor_scalar_add(oT[:], p4[:], b4s)
            nc.sync.dma_start(yout[:, c * NB:(c + 1) * NB], oT[:])

        def full_pass(it=0):
            if it != 0:
                dma_in(0, it)
            for m in range(KT):
                l1_group(0, m, it)
            for c in range(nch):
                if c + 1 < nch:
                    dma_in(c + 1, it)
                for m in range(KT):
                    mid_group(2, w2s, hAs[c % 2], hBs[c % 2], c, m, it)
                for m in range(KT):
                    mid_group(3, w3s, hBs[c % 2], hCs[c % 2], c, m, it)
                    if c + 1 < nch:
                        l1_group(c + 1, m, it)
                l4_chunk(c, it)

        if iters == 1:
            full_pass()
        else:
            with tc.For_i(0, iters, 1):
                full_pass()

    nc.compile()
    return nc


def _prep_in_maps(t, u, W1, b1, W2, b2, W3, b3, W4, b4):
    in_maps = []
    for e in range(N_ENV):
        in_maps.append({
            "xt": u[e::N_ENV].T.astype(NP_BF16, order='C'),
            "w1": W1[e].astype(NP_BF16, order='C'),
            "w2": W2[e].reshape(KT, 128, H).transpose(1, 0, 2)
                  .astype(NP_BF16, order='C'),
            "w3": W3[e].reshape(KT, 128, H).transpose(1, 0, 2)
                  .astype(NP_BF16, order='C'),
            "w4": W4[e].reshape(KT, 128, D).transpose(1, 0, 2)
                  .astype(NP_BF16, order='C'),
            "b1": np.ascontiguousarray(b1[e].reshape(KT, 128).T),
            "b2": np.ascontiguousarray(b2[e].reshape(KT, 128).T),
            "b3": np.ascontiguousarray(b3[e].reshape(KT, 128).T),
            "b4": np.ascontiguousarray(b4[e].reshape(D, 1)),
        })
    return in_maps


_CACHED_NC = None


def kernel(t, u, W1, b1, W2, b2, W3, b3, W4, b4):
    global _CACHED_NC
    u = np.asarray(u, np.float32)
    args = [np.asarray(a, np.float32) for a in (W1, b1, W2, b2, W3, b3, W4, b4)]
    if _CACHED_NC is None:
        _CACHED_NC = build_module()
    in_maps = _prep_in_maps(None, u, *args)
    res = run_bass_kernel_spmd(_CACHED_NC, in_maps, core_ids=list(range(N_ENV)))
    out = np.empty((B * N_ENV, D), np.float32)
    for e in range(N_ENV):
        out[e::N_ENV] = res.results[e]["y"].T
    return out


# revision 14
# speedup vs baseline: 1.0402x; 1.0402x over previous
"""Per-env MLP (EnvironVectorField) Trainium2 kernel.

Reference computation (fp32):
    x = u.reshape(B, E, D)  # B=16384, E=8 envs, D=64
    h = swish(x @ W1[e] + b1[e]); h = swish(h @ W2[e] + b2[e])
    h = swish(h @ W3[e] + b3[e]); out = h @ W4[e] + b4[e]
    return out.reshape(B*E, D)

Sharding: expert-parallel — core e computes env e entirely (u rows e::8).

Per-core design: activations are feature-major (features on SBUF
partitions, batch on the free axis) so weights are the stationary matmul
operand. The input/output transposes are done on the HOST (numpy), so the
device only runs matmuls + activations.

Dtypes: everything bf16 (weights host-converted, activations written
bf16 by the PSUM-evacuating Silu). Accumulation is fp32 in PSUM.
bf16 streams at 1 cycle/row on the PE like f32r, but halves DMA bytes
and SBUF footprint. (fp8+DoubleRow was evaluated: 2x streaming rate but
4.2e-2 rel err -- over the 2e-2 gate. bf16 measures 3.6e-3.)
"""

import sys

sys.path.insert(0, '/opt/trn_rl_repo')

from contextlib import ExitStack

import ml_dtypes
import numpy as np

import concourse.bacc as bacc
import concourse.bass as bass
import concourse.mybir as mybir
import concourse.tile as tile
from concourse.bass_utils import run_bass_kernel_spmd

F32 = mybir.dt.float32
BF16 = mybir.dt.bfloat16
SILU = mybir.ActivationFunctionType.Silu

NP_BF16 = ml_dtypes.bfloat16

N_ENV = 8
D = 64          # state dim
H = 1024        # hidden dim
B = 16384       # rows per env
NB = 1024       # batch-chunk columns per chunk
NCH = B // NB   # 16 chunks
NT = NB // 512  # 512-wide matmul n-tiles per chunk
KT = H // 128   # 8 k/m tiles of 128 over the hidden dim


def build_module(iters: int = 1, nch: int = NCH):
    nc = bacc.Bacc("TRN2", target_bir_lowering=False, num_devices=N_ENV)

    xin = nc.dram_tensor("xt", (D, nch * NB), BF16, kind="ExternalInput")
    w1d = nc.dram_tensor("w1", (128, H), BF16, kind="ExternalInput")
    w2d = nc.dram_tensor("w2", (128, KT, H), BF16, kind="ExternalInput")
    w3d = nc.dram_tensor("w3", (128, KT, H), BF16, kind="ExternalInput")
    w4d = nc.dram_tensor("w4", (128, KT, D), BF16, kind="ExternalInput")
    b1d = nc.dram_tensor("b1", (128, KT), F32, kind="ExternalInput")
    b2d = nc.dram_tensor("b2", (128, KT), F32, kind="ExternalInput")
    b3d = nc.dram_tensor("b3", (128, KT), F32, kind="ExternalInput")
    b4d = nc.dram_tensor("b4", (D, 1), F32, kind="ExternalInput")
    yout = nc.dram_tensor("y", (D, nch * NB), F32, kind="ExternalOutput")

    with tile.TileContext(nc) as tc, ExitStack() as ctx:
        wpool = ctx.enter_context(tc.tile_pool(name="wpool", bufs=1))
        mps = ctx.enter_context(tc.tile_pool(name="mps", bufs=3, space="PSUM"))
        mp4 = ctx.enter_context(tc.tile_pool(name="mp4", bufs=1, space="PSUM"))

        # persistent tiles (xt/w1 carry x duplicated on partitions 64:128
        # so L1 runs as two concurrent K=64 row-group matmuls)
        xts = [wpool.tile([128, NB], BF16, name=f"xt{i}") for i in range(2)]
        w1s = wpool.tile([128, H], BF16)
        w2s = wpool.tile([128, KT, H], BF16)
        w3s = wpool.tile([128, KT, H], BF16)
        w4s = wpool.tile([128, KT, D], BF16)
        ball = wpool.tile([128, 3 * KT + 1], F32)
        hAs = [wpool.tile([128, KT, NB], BF16, name=f"hA{i}") for i in range(2)]
        hBs = [wpool.tile([128, KT, NB], BF16, name=f"hB{i}") for i in range(2)]
        hCs = [wpool.tile([128, KT, NB], BF16, name=f"hC{i}") for i in range(2)]
        oTs = [wpool.tile([D, NB], F32, name=f"oT{i}") for i in range(2)]

        # chunk-0 input first (FIFO DMA queues), then weights/biases
        nc.sync.dma_start(xts[0][:D], xin[:, 0:NB])
        nc.sync.dma_start(xts[0][D:128], xin[:, 0:NB])
        nc.sync.dma_start(w1s[:], w1d[:])
        nc.sync.dma_start(ball[:, 0:KT], b1d[:])
        nc.sync.dma_start(ball[:, KT:2 * KT], b2d[:])
        nc.sync.dma_start(ball[:, 2 * KT:3 * KT], b3d[:])
        nc.sync.dma_start(ball[:D, 3 * KT:3 * KT + 1], b4d[:])
        nc.sync.dma_start(ball[D:128, 3 * KT:3 * KT + 1], b4d[:])
        nc.sync.dma_start(w2s[:], w2d[:])
        nc.sync.dma_start(w3s[:], w3d[:])
        nc.sync.dma_start(w4s[:], w4d[:])
        b4lo = ball[:D, 3 * KT:3 * KT + 1]
        b4hi = ball[D:128, 3 * KT:3 * KT + 1]

        def dma_in(c, it=0):
            xt = xts[c % 2]
            src = xin[:, c * NB:(c + 1) * NB]
            nc.sync.dma_start(xt[:D], src)
            nc.sync.dma_start(xt[D:128], src)

        def l1_group(c, mp, it=0):
            # row-packed: m-tiles 2mp (rows 0:64) and 2mp+1 (rows 64:128)
            # run concurrently in distinct PE row groups
            xt = xts[c % 2]
            hA = hAs[c % 2]
            mA, mB = 2 * mp, 2 * mp + 1
            pa = mps.tile([128, NB], F32, tag="mmw", name=f"p1a_{it}_{c}_{mp}")
            pb = mps.tile([128, NB], F32, tag="mmw", name=f"p1b_{it}_{c}_{mp}")
            for n in range(NT):
                ns_ = slice(n * 512, (n + 1) * 512)
                nc.tensor.matmul(pa[:, ns_], w1s[:D, mA * 128:(mA + 1) * 128],
                                 xt[:D, ns_], start=True, stop=True,
                                 tile_position=(0, 0))
                nc.tensor.matmul(pb[:, ns_], w1s[D:128, mB * 128:(mB + 1) * 128],
                                 xt[D:128, ns_], start=True, stop=True,
                                 tile_position=(64, 0))
            nc.scalar.activation(hA[:, mA, :], pa[:], SILU,
                                 bias=ball[:, mA:mA + 1])
            nc.scalar.activation(hA[:, mB, :], pb[:], SILU,
                                 bias=ball[:, mB:mB + 1])

        def mid_group(li, ws, hs, hd, c, m, it=0):
            pm = mps.tile([128, NB], F32, tag="mmw", name=f"p{li}_{it}_{c}_{m}")
            for k in range(KT):
                for n in range(NT):
                    ns_ = slice(n * 512, (n + 1) * 512)
                    nc.tensor.matmul(pm[:, ns_],
                                     ws[:, k, m * 128:(m + 1) * 128],
                                     hs[:, k, ns_],
                                     start=(k == 0), stop=(k == KT - 1))
            bcol = (li - 1) * KT + m
            nc.scalar.activation(hd[:, m, :], pm[:], SILU,
                                 bias=ball[:, bcol:bcol + 1])

        def l4_chunk(c, it=0):
            # col-packed: batch halves 0:512 / 512:1024 land on psum
            # partitions 0:64 / 64:128 in distinct PE column groups
            hC = hCs[c % 2]
            oT = oTs[c % 2]
            p4 = mp4.tile([128, 512], F32, tag="p4", name=f"p4_{it}_{c}")
            for k in range(KT):
                nc.tensor.matmul(p4[:D, :], w4s[:, k, :], hC[:, k, 0:512],
                                 start=(k == 0), stop=(k == KT - 1),
                                 tile_position=(0, 0), skip_group_check=True)
                nc.tensor.matmul(p4[D:128, :], w4s[:, k, :], hC[:, k, 512:1024],
                                 start=(k == 0), stop=(k == KT - 1),
                                 tile_position=(0, 64), skip_group_check=True)
            nc.vector.tensor_scalar_add(oT[:, 0:512], p4[:D, :], b4lo)
            nc.vector.tensor_scalar_add(oT[:, 512:1024], p4[D:128, :], b4hi)
            nc.sync.dma_start(yout[:, c * NB:(c + 1) * NB], oT[:])

        def full_pass(it=0):
            if it != 0:
                dma_in(0, it)
            for mp in range(KT // 2):
                l1_group(0, mp, it)
            for c in range(nch):
                if c + 1 < nch:
                    dma_in(c + 1, it)
                for m in range(KT):
                    mid_group(2, w2s, hAs[c % 2], hBs[c % 2], c, m, it)
                for m in range(KT):
                    mid_group(3, w3s, hBs[c % 2], hCs[c % 2], c, m, it)
                    if c + 1 < nch and m % 2 == 1:
                        l1_group(c + 1, m // 2, it)
                l4_chunk(c, it)

        if iters == 1:
            full_pass()
        else:
            with tc.For_i(0, iters, 1):
                full_pass()

    nc.compile()
    return nc


def _prep_in_maps(t, u, W1, b1, W2, b2, W3, b3, W4, b4):
    in_maps = []
    for e in range(N_ENV):
        in_maps.append({
            "xt": u[e::N_ENV].T.astype(NP_BF16, order='C'),
            "w1": np.vstack([W1[e], W1[e]]).astype(NP_BF16, order='C'),
            "w2": W2[e].reshape(KT, 128, H).transpose(1, 0, 2)
                  .astype(NP_BF16, order='C'),
            "w3": W3[e].reshape(KT, 128, H).transpose(1, 0, 2)
                  .astype(NP_BF16, order='C'),
            "w4": W4[e].reshape(KT, 128, D).transpose(1, 0, 2)
                  .astype(NP_BF16, order='C'),
            "b1": np.ascontiguousarray(b1[e].reshape(KT, 128).T),
            "b2": np.ascontiguousarray(b2[e].reshape(KT, 128).T),
            "b3": np.ascontiguousarray(b3[e].reshape(KT, 128).T),
            "b4": np.ascontiguousarray(b4[e].reshape(D, 1)),
        })
    return in_maps


_CACHED_NC = None


def kernel(t, u, W1, b1, W2, b2, W3, b3, W4, b4):
    global _CACHED_NC
    u = np.asarray(u, np.float32)
    args = [np.asarray(a, np.float32) for a in (W1, b1, W2, b2, W3, b3, W4, b4)]
    if _CACHED_NC is None:
        _CACHED_NC = build_module()
    in_maps = _prep_in_maps(None, u, *args)
    res = run_bass_kernel_spmd(_CACHED_NC, in_maps, core_ids=list(range(N_ENV)))
    out = np.empty((B * N_ENV, D), np.float32)
    for e in range(N_ENV):
        out[e::N_ENV] = res.results[e]["y"].T
    return out


# revision 20
# speedup vs baseline: 1.0679x; 1.0266x over previous
"""Per-env MLP (EnvironVectorField) Trainium2 kernel.

Reference computation (fp32):
    x = u.reshape(B, E, D)  # B=16384, E=8 envs, D=64
    h = swish(x @ W1[e] + b1[e]); h = swish(h @ W2[e] + b2[e])
    h = swish(h @ W3[e] + b3[e]); out = h @ W4[e] + b4[e]
    return out.reshape(B*E, D)

Sharding: expert-parallel — core e computes env e entirely (u rows e::8).

Per-core design: activations are feature-major (features on SBUF
partitions, batch on the free axis) so weights are the stationary matmul
operand. The input/output transposes are done on the HOST (numpy), so the
device only runs matmuls + activations.

Dtypes: everything bf16 (weights host-converted, activations written
bf16 by the PSUM-evacuating Silu). Accumulation is fp32 in PSUM.
bf16 streams at 1 cycle/row on the PE like f32r, but halves DMA bytes
and SBUF footprint. (fp8+DoubleRow was evaluated: 2x streaming rate but
4.2e-2 rel err -- over the 2e-2 gate. bf16 measures 3.6e-3.)
"""

import sys

sys.path.insert(0, '/opt/trn_rl_repo')

from contextlib import ExitStack

import ml_dtypes
import numpy as np

import concourse.bacc as bacc
import concourse.bass as bass
import concourse.mybir as mybir
import concourse.tile as tile
from concourse.bass_utils import run_bass_kernel_spmd

F32 = mybir.dt.float32
BF16 = mybir.dt.bfloat16
SILU = mybir.ActivationFunctionType.Silu

NP_BF16 = ml_dtypes.bfloat16

N_ENV = 8
D = 64          # state dim
H = 1024        # hidden dim
B = 16384       # rows per env
NB = 1024       # batch-chunk columns per chunk
NCH = B // NB   # 16 chunks
NT = NB // 512  # 512-wide matmul n-tiles per chunk
KT = H // 128   # 8 k/m tiles of 128 over the hidden dim


def build_module(iters: int = 1, nch: int = NCH):
    nc = bacc.Bacc("TRN2", target_bir_lowering=False, num_devices=N_ENV)

    xin = nc.dram_tensor("xt", (128, nch * NB), BF16, kind="ExternalInput")
    w1d = nc.dram_tensor("w1", (128, H), BF16, kind="ExternalInput")
    w2d = nc.dram_tensor("w2", (128, KT, H), BF16, kind="ExternalInput")
    w3d = nc.dram_tensor("w3", (128, KT, H), BF16, kind="ExternalInput")
    w4d = nc.dram_tensor("w4", (128, KT, D), BF16, kind="ExternalInput")
    b1d = nc.dram_tensor("b1", (128, KT), F32, kind="ExternalInput")
    b2d = nc.dram_tensor("b2", (128, KT), F32, kind="ExternalInput")
    b3d = nc.dram_tensor("b3", (128, KT), F32, kind="ExternalInput")
    b4d = nc.dram_tensor("b4", (D, 1), F32, kind="ExternalInput")
    yout = nc.dram_tensor("y", (D, nch * NB), F32, kind="ExternalOutput")

    with tile.TileContext(nc) as tc, ExitStack() as ctx:
        wpool = ctx.enter_context(tc.tile_pool(name="wpool", bufs=1))
        mps = ctx.enter_context(tc.tile_pool(name="mps", bufs=4, space="PSUM"))

        # persistent tiles (xt/w1 carry x duplicated on partitions 64:128
        # so L1 runs as two concurrent K=64 row-group matmuls)
        xts = [wpool.tile([128, NB], BF16, name=f"xt{i}") for i in range(2)]
        w1s = wpool.tile([128, H], BF16)
        w2s = wpool.tile([128, KT, H], BF16)
        w3s = wpool.tile([128, KT, H], BF16)
        w4s = wpool.tile([128, KT, D], BF16)
        ball = wpool.tile([128, 3 * KT + 1], F32)
        hAs = [wpool.tile([128, KT, NB], BF16, name=f"hA{i}") for i in range(2)]
        hBs = [wpool.tile([128, KT, NB], BF16, name=f"hB{i}") for i in range(2)]
        hCs = [wpool.tile([128, KT, NB], BF16, name=f"hC{i}") for i in range(2)]
        oTs = [wpool.tile([D, NB], F32, name=f"oT{i}") for i in range(2)]

        # chunk-0 input first (FIFO DMA queues), then weights/biases
        nc.sync.dma_start(xts[0][:], xin[:, 0:NB])
        nc.sync.dma_start(w1s[:], w1d[:])
        nc.sync.dma_start(ball[:, 0:KT], b1d[:])
        nc.sync.dma_start(ball[:, KT:2 * KT], b2d[:])
        nc.sync.dma_start(ball[:, 2 * KT:3 * KT], b3d[:])
        nc.sync.dma_start(ball[:D, 3 * KT:3 * KT + 1], b4d[:])
        nc.sync.dma_start(ball[D:128, 3 * KT:3 * KT + 1], b4d[:])
        nc.sync.dma_start(w2s[:], w2d[:])
        nc.sync.dma_start(w3s[:], w3d[:])
        nc.sync.dma_start(w4s[:], w4d[:])
        b4lo = ball[:D, 3 * KT:3 * KT + 1]
        b4hi = ball[D:128, 3 * KT:3 * KT + 1]

        def dma_in(c, it=0):
            nc.sync.dma_start(xts[c % 2][:], xin[:, c * NB:(c + 1) * NB])

        def l1_group(c, mp, it=0):
            # row-packed: m-tiles 2mp (rows 0:64) and 2mp+1 (rows 64:128)
            # run concurrently in distinct PE row groups
            xt = xts[c % 2]
            hA = hAs[c % 2]
            mA, mB = 2 * mp, 2 * mp + 1
            pa = mps.tile([128, NB], F32, tag="mmw", name=f"p1a_{it}_{c}_{mp}")
            pb = mps.tile([128, NB], F32, tag="mmw", name=f"p1b_{it}_{c}_{mp}")
            for n in range(NT):
                ns_ = slice(n * 512, (n + 1) * 512)
                nc.tensor.matmul(pa[:, ns_], w1s[:D, mA * 128:(mA + 1) * 128],
                                 xt[:D, ns_], start=True, stop=True,
                                 tile_position=(0, 0))
                nc.tensor.matmul(pb[:, ns_], w1s[D:128, mB * 128:(mB + 1) * 128],
                                 xt[D:128, ns_], start=True, stop=True,
                                 tile_position=(64, 0))
            nc.scalar.activation(hA[:, mA, :], pa[:], SILU,
                                 bias=ball[:, mA:mA + 1])
            nc.scalar.activation(hA[:, mB, :], pb[:], SILU,
                                 bias=ball[:, mB:mB + 1])

        def mid_group(li, ws, hs, hd, c, m, it=0):
            pm = mps.tile([128, NB], F32, tag="mmw", name=f"p{li}_{it}_{c}_{m}")
            for k in range(KT):
                for n in range(NT):
                    ns_ = slice(n * 512, (n + 1) * 512)
                    nc.tensor.matmul(pm[:, ns_],
                                     ws[:, k, m * 128:(m + 1) * 128],
                                     hs[:, k, ns_],
                                     start=(k == 0), stop=(k == KT - 1))
            bcol = (li - 1) * KT + m
            nc.scalar.activation(hd[:, m, :], pm[:], SILU,
                                 bias=ball[:, bcol:bcol + 1])

        def l4_chunk(c, it=0):
            # col-packed: batch halves 0:512 / 512:1024 land on psum
            # partitions 0:64 / 64:128 in distinct PE column groups
            hC = hCs[c % 2]
            oT = oTs[c % 2]
            p4 = mps.tile([128, NB], F32, tag="mmw", name=f"p4_{it}_{c}")
            for k in range(KT):
                nc.tensor.matmul(p4[:D, :512], w4s[:, k, :], hC[:, k, 0:512],
                                 start=(k == 0), stop=(k == KT - 1),
                                 tile_position=(0, 0), skip_group_check=True)
                nc.tensor.matmul(p4[D:128, :512], w4s[:, k, :], hC[:, k, 512:1024],
                                 start=(k == 0), stop=(k == KT - 1),
                                 tile_position=(0, 64), skip_group_check=True)
            nc.vector.tensor_scalar_add(oT[:, 0:512], p4[:D, :512], b4lo)
            nc.sync.dma_start(yout[:, c * NB:c * NB + 512], oT[:, 0:512])
            nc.vector.tensor_scalar_add(oT[:, 512:1024], p4[D:128, :512], b4hi)
            nc.sync.dma_start(yout[:, c * NB + 512:(c + 1) * NB], oT[:, 512:1024])

        def full_pass(it=0):
            if it != 0:
                dma_in(0, it)
            for mp in range(KT // 2):
                l1_group(0, mp, it)
            for c in range(nch):
                if c + 1 < nch:
                    dma_in(c + 1, it)
                for m in range(KT):
                    mid_group(2, w2s, hAs[c % 2], hBs[c % 2], c, m, it)
                for m in range(KT):
                    mid_group(3, w3s, hBs[c % 2], hCs[c % 2], c, m, it)
                    if c + 1 < nch and m % 2 == 1:
                        l1_group(c + 1, m // 2, it)
                l4_chunk(c, it)

        if iters == 1:
            full_pass()
        else:
            with tc.For_i(0, iters, 1):
                full_pass()

    nc.compile()
    return nc


def _prep_in_maps(t, u, W1, b1, W2, b2, W3, b3, W4, b4):
    in_maps = []
    for e in range(N_ENV):
        in_maps.append({
            "xt": np.vstack([xt8 := u[e::N_ENV].T.astype(NP_BF16, order='C'),
                             xt8]),
            "w1": np.vstack([W1[e], W1[e]]).astype(NP_BF16, order='C'),
            "w2": W2[e].reshape(KT, 128, H).transpose(1, 0, 2)
                  .astype(NP_BF16, order='C'),
            "w3": W3[e].reshape(KT, 128, H).transpose(1, 0, 2)
                  .astype(NP_BF16, order='C'),
            "w4": W4[e].reshape(KT, 128, D).transpose(1, 0, 2)
                  .astype(NP_BF16, order='C'),
            "b1": np.ascontiguousarray(b1[e].reshape(KT, 128).T),
            "b2": np.ascontiguousarray(b2[e].reshape(KT, 128).T),
            "b3": np.ascontiguousarray(b3[e].reshape(KT, 128).T),
            "b4": np.ascontiguousarray(b4[e].reshape(D, 1)),
        })
    return in_maps


_CACHED_NC = None


def kernel(t, u, W1, b1, W2, b2, W3, b3, W4, b4):
    global _CACHED_NC
    u = np.asarray(u, np.float32)
    args = [np.asarray(a, np.float32) for a in (W1, b1, W2, b2, W3, b3, W4, b4)]
    if _CACHED_NC is None:
        _CACHED_NC = build_module()
    in_maps = _prep_in_maps(None, u, *args)
    res = run_bass_kernel_spmd(_CACHED_NC, in_maps, core_ids=list(range(N_ENV)))
    out = np.empty((B * N_ENV, D), np.float32)
    for e in range(N_ENV):
        out[e::N_ENV] = res.results[e]["y"].T
    return out
